# revision 45
# baseline (speedup 1.0000x reference)
"""PIoU (pixel-wise IoU) pairwise matrix kernel for Trainium2, 8 NeuronCores.

Math: for each pair (predicted box n, target box m) the reference samples a
16x16 grid of the joint AABB and evaluates a soft membership
F = sigmoid(k(w/2-|A|)) * sigmoid(k(h/2-|B|)) per box, where (A, B) are the
pixel offsets rotated into the box frame.  Both A and B are *affine* in the
grid coordinates (ug, uh), so a K=12 fp32 matmul per (box, 128-target
chunk) against a constant (1, Ug, Uh) basis emits R = K*A for all 256
pixels x 4 fields; |R| is one DVE sign-bit clear, and ACT evaluates
sigma(kh - |R|) with the per-field kh as per-partition bias (scale=-1).
Products and reductions are spread over DVE (F1,F2 + Ssum), Pool (F1*F2),
and ACT (Isum via a Copy activation's accum_out) so the PE stays busy.

Sharding: N (predicted) axis split 8 ways; each core computes a [512m, 64n]
slab (output transposed on host).

Dispatch: the per-call cost is dominated by the axon tunnel round trip
(~80ms) plus payload bytes, so the runner (a) jits the PJRT shard_map
function ONCE and reuses it every call, (b) keeps the constant basis and
the structural zero-output buffers resident on device, and (c) ships only
one packed [144,40] f32 tensor per core per call (~184KB total) with the
per-box broadcast done on device via a K=1 matmul against an all-ones
basis row.  Output returns as fp16 (512KB) and is upcast on host.
"""

import numpy as np

N = 512
M = 512
G = 16
NPIX = G * G
K_SLOPE = np.float32(10.0)
EPS = np.float32(1e-6)
NC = 8
NLOC = N // NC  # 64 predicted boxes per core
NCHUNK = 4  # m-chunks of 128
XROWS = 144  # packed input: rows 0..127 = TQ [128,40], rows 128..143 = PB [16,40]

_cache = {}

_Q_ORDER = ("x0", "x1", "y0", "y1", "cx", "cy", "ct", "st", "khw", "khh")


def _derived(b):
    # b: [K,5] float32 -> per-box derived quantities (all float32)
    cx, cy, w, h, t = (b[:, i].astype(np.float32) for i in range(5))
    c, s = np.cos(t).astype(np.float32), np.sin(t).astype(np.float32)
    hw = np.float32(0.5) * (w * np.abs(c) + h * np.abs(s))
    hh = np.float32(0.5) * (w * np.abs(s) + h * np.abs(c))
    return dict(
        cx=cx, cy=cy, ct=c, st=s,
        khw=(K_SLOPE * np.float32(0.5)) * w, khh=(K_SLOPE * np.float32(0.5)) * h,
        x0=cx - hw, x1=cx + hw, y0=cy - hh, y1=cy + hh,
    )


def _basis():
    """Constant [12, NPIX*4] sampling basis (shipped to device once).

    Field f (A1, A2, B1, B2) occupies cols f*NPIX..(f+1)*NPIX and uses rows
    3f..3f+2 as (1, Ug, Uh).  The matmul emits R = K*(a0 + a1*ug + a2*uh)
    per field; |R| (sign-bit clear on DVE) then feeds sigma(kh - |R|) on
    ACT via per-field per-partition biases.  Row 0 is all-ones over cols
    0..255; its first 128 cols double as the ones-lhsT for the on-device
    partition broadcast of the per-box rows.
    """
    if "basis" in _cache:
        return _cache["basis"]
    u = ((np.arange(G, dtype=np.float32) + np.float32(0.5)) / np.float32(G))
    Ug = np.tile(u, G)      # pixel p = h*G+g -> u[g]
    Uh = np.repeat(u, G)    # -> u[h]

    basis = np.zeros((12, 4 * NPIX), dtype=np.float32)
    for f in range(4):
        c0 = f * NPIX
        basis[3 * f + 0, c0:c0 + NPIX] = 1.0
        basis[3 * f + 1, c0:c0 + NPIX] = Ug
        basis[3 * f + 2, c0:c0 + NPIX] = Uh
    _cache["basis"] = basis
    return basis


def _pack_inputs(loc_p, loc_t):
    """Per-call packed input [NC*XROWS, 40]: TQ (replicated) + per-core PB."""
    T = _derived(loc_t)
    # TQ [128, 4 chunks, 10]: per-target quantities, m = j*128 + partition
    TQ = np.empty((128, NCHUNK, len(_Q_ORDER)), dtype=np.float32)
    for qi, q in enumerate(_Q_ORDER):
        TQ[:, :, qi] = T[q].reshape(NCHUNK, 128).T
    TQflat = TQ.reshape(128, NCHUNK * len(_Q_ORDER))

    P = _derived(loc_p)
    X = np.empty((NC, XROWS, 40), dtype=np.float32)
    for c in range(NC):
        sl = slice(c * NLOC, (c + 1) * NLOC)
        pb = np.stack([P[q][sl] for q in _Q_ORDER], axis=0)  # [10, 64]
        X[c, 0:128, :] = TQflat
        X[c, 128:XROWS, :] = pb.reshape(16, 40)
    return X.reshape(NC * XROWS, 40)


def _build_nc():
    from contextlib import ExitStack

    import concourse.bacc as bacc
    import concourse.tile as tile
    from concourse import mybir
    from concourse.masks import make_identity

    dt = mybir.dt
    op = mybir.AluOpType
    AF = mybir.ActivationFunctionType
    K = float(K_SLOPE)

    # Bacc (not raw Bass): its finalize() runs generate_event_semaphores,
    # which legalizes Tile's multi-wait sync_info down to <=1 wait per
    # hardware instruction.
    nc = bacc.Bacc(None, target_bir_lowering=False)
    X_d = nc.declare_dram_parameter("X", [XROWS, 40], dt.float32, isOutput=False)
    BAS_d = nc.declare_dram_parameter("BASIS", [12, 4 * NPIX], dt.float32, isOutput=False)
    OUT_d = nc.declare_dram_parameter("OUT", [M, NLOC], dt.float16, isOutput=True)

    with tile.TileContext(nc) as tc, ExitStack() as ctx:
        consts = ctx.enter_context(tc.tile_pool(name="consts", bufs=1))
        coeffp = ctx.enter_context(tc.tile_pool(name="coeffp", bufs=2))
        scratch = ctx.enter_context(tc.tile_pool(name="scratch", bufs=2))
        work = ctx.enter_context(tc.tile_pool(name="work", bufs=2))
        accp = ctx.enter_context(tc.tile_pool(name="accp", bufs=2))
        psum = ctx.enter_context(tc.tile_pool(name="psum", bufs=2, space="PSUM"))

        ident = consts.tile([128, 128], dt.float32)
        make_identity(nc, ident[:])
        TQ = consts.tile([128, NCHUNK, 10], dt.float32)
        nc.sync.dma_start(out=TQ[:].rearrange("p a b -> p (a b)"), in_=X_d[0:128, :])
        pbrow = consts.tile([1, 10 * NLOC], dt.float32)
        nc.sync.dma_start(
            out=pbrow[:],
            in_=X_d[128:XROWS, :].rearrange("a b -> (a b)").unsqueeze(0))
        BAS = consts.tile([12, 4 * NPIX], dt.float32)
        nc.sync.dma_start(out=BAS[:], in_=BAS_d[:])

        # Broadcast the 640 per-box values to all 128 partitions with a K=1
        # matmul: BAS row 0 is all-ones on cols 0..255, so BAS[0:1, 0:128]
        # is a ready-made ones lhsT.  PSUM reuses one "fields" ring slot.
        PB = consts.tile([128, 10, NLOC], dt.float32)
        PBflat = PB[:].rearrange("p a b -> p (a b)")
        bc = psum.tile([128, 2 * NPIX], dt.float32, tag="fieldA", bufs=3)
        nc.tensor.matmul(bc[:], BAS[0:1, 0:128],
                         pbrow[:, 0:512].broadcast_to([1, 512]),
                         start=True, stop=True)
        bc2 = psum.tile([128, 2 * NPIX], dt.float32, tag="fieldB", bufs=3)
        nc.tensor.matmul(bc2[:, 0:128], BAS[0:1, 0:128], pbrow[:, 512:640],
                         start=True, stop=True)
        nc.vector.tensor_copy(PBflat[:, 0:512], bc[:])
        nc.vector.tensor_copy(PBflat[:, 512:640], bc2[:, 0:128])

        def pb(q):
            return PB[:, _Q_ORDER.index(q), :]

        def tq(j, q):
            i = _Q_ORDER.index(q)
            return TQ[:, j, i:i + 1]

        for j in range(NCHUNK):
            # ---- coefficient slab C [128 m, 64 n, 12 rows] on DVE ----
            C = coeffp.tile([128, NLOC, 12], dt.float32, tag="C")
            S = scratch.tile([128, 16, NLOC], dt.float32, tag="S")
            g = nc.vector

            def s(i):
                return S[:, i, :]

            g.tensor_scalar(s(0), pb("x0"), tq(j, "x0"), None, op.min)   # xmin
            g.tensor_scalar(s(1), pb("x1"), tq(j, "x1"), None, op.max)   # xmax
            g.tensor_scalar(s(2), pb("y0"), tq(j, "y0"), None, op.min)   # ymin
            g.tensor_scalar(s(3), pb("y1"), tq(j, "y1"), None, op.max)   # ymax
            g.tensor_tensor(s(4), s(1), s(0), op.subtract)               # sx
            g.tensor_tensor(s(5), s(3), s(2), op.subtract)               # sy
            g.tensor_tensor(s(6), s(0), pb("cx"), op.subtract)           # dxp
            g.tensor_tensor(s(7), s(2), pb("cy"), op.subtract)           # dyp
            # a0p = dxp*ctp + dyp*stp ; b0p = dyp*ctp - dxp*stp
            g.tensor_tensor(s(8), s(6), pb("ct"), op.mult)
            g.tensor_tensor(s(9), s(7), pb("st"), op.mult)
            g.tensor_tensor(s(9), s(8), s(9), op.add)                    # a0p
            g.tensor_tensor(s(8), s(7), pb("ct"), op.mult)
            g.tensor_tensor(s(10), s(6), pb("st"), op.mult)
            g.tensor_tensor(s(10), s(8), s(10), op.subtract)             # b0p

            def c(r):
                return C[:, :, r]

            # R-rows: field f cols use basis rows 3f..3f+2 as (1, Ug, Uh);
            # coefficient rows are K*(a0, a1, a2) per field.
            # field A1 (pred w-axis): K*a0p, K*a1p (=K*sx*ctp), K*a2p (=K*sy*stp)
            g.tensor_scalar(c(0), s(9), K, None, op.mult)
            g.tensor_tensor(s(8), s(4), pb("ct"), op.mult)
            g.tensor_scalar(c(1), s(8), K, None, op.mult)
            g.tensor_tensor(s(8), s(5), pb("st"), op.mult)
            g.tensor_scalar(c(2), s(8), K, None, op.mult)
            # field B1 (pred h-axis, rows 6-8): K*b0p, -K*sx*stp, K*sy*ctp
            g.tensor_scalar(c(6), s(10), K, None, op.mult)
            g.tensor_tensor(s(8), s(4), pb("st"), op.mult)
            g.tensor_scalar(c(7), s(8), -K, None, op.mult)
            g.tensor_tensor(s(8), s(5), pb("ct"), op.mult)
            g.tensor_scalar(c(8), s(8), K, None, op.mult)
            # target box: dxt/dyt
            g.tensor_scalar(s(12), s(0), tq(j, "cx"), None, op.subtract)
            g.tensor_scalar(s(13), s(2), tq(j, "cy"), None, op.subtract)
            # a0t = dxt*ctt + dyt*stt
            g.tensor_scalar(s(8), s(12), tq(j, "ct"), None, op.mult)
            g.tensor_scalar(s(14), s(13), tq(j, "st"), None, op.mult)
            g.tensor_tensor(s(14), s(8), s(14), op.add)
            # b0t = dyt*ctt - dxt*stt
            g.tensor_scalar(s(8), s(13), tq(j, "ct"), None, op.mult)
            g.tensor_scalar(s(15), s(12), tq(j, "st"), None, op.mult)
            g.tensor_tensor(s(15), s(8), s(15), op.subtract)
            # field A2 (tgt w-axis, rows 3-5): K*a0t, K*sx*ctt, K*sy*stt
            g.tensor_scalar(c(3), s(14), K, None, op.mult)
            g.tensor_scalar(s(8), s(4), tq(j, "ct"), None, op.mult)
            g.tensor_scalar(c(4), s(8), K, None, op.mult)
            g.tensor_scalar(s(8), s(5), tq(j, "st"), None, op.mult)
            g.tensor_scalar(c(5), s(8), K, None, op.mult)
            # field B2 (tgt h-axis, rows 9-11): K*b0t, -K*sx*stt, K*sy*ctt
            g.tensor_scalar(c(9), s(15), K, None, op.mult)
            g.tensor_scalar(s(8), s(4), tq(j, "st"), None, op.mult)
            g.tensor_scalar(c(10), s(8), -K, None, op.mult)
            g.tensor_scalar(s(8), s(5), tq(j, "ct"), None, op.mult)
            g.tensor_scalar(c(11), s(8), K, None, op.mult)

            Ssum = accp.tile([128, NLOC], dt.float32, tag="Ssum")
            Isum = accp.tile([128, NLOC], dt.float32, tag="Isum")

            # ---- main loop over the 64 predicted boxes of this core ----
            for n in range(NLOC):
                coeffT = psum.tile([12, 128], dt.float32, tag="coeffT")
                nc.tensor.transpose(coeffT[:], C[:, n, :], ident[:])
                lhsT = work.tile([12, 128], dt.float32, tag="lhsT", bufs=3)
                nc.vector.tensor_copy(lhsT[:], coeffT[:])

                fieldA = psum.tile([128, 2 * NPIX], dt.float32, tag="fieldA", bufs=3)
                fieldB = psum.tile([128, 2 * NPIX], dt.float32, tag="fieldB", bufs=3)
                nc.tensor.matmul(fieldA[:], lhsT[:], BAS[:, 0:512],
                                 start=True, stop=True)
                nc.tensor.matmul(fieldB[:], lhsT[:], BAS[:, 512:1024],
                                 start=True, stop=True)
                # |R| by clearing the sign bit (single PSUM read; GPSIMD
                # cannot access PSUM and DVE allows only one PSUM input).
                # Two halves so the A-half abs overlaps the B-half matmul.
                absR = work.tile([128, 4 * NPIX], dt.float32, tag="absR", bufs=3)
                nc.vector.tensor_scalar(
                    absR[:, 0:2 * NPIX].bitcast(dt.int32),
                    fieldA[:].bitcast(dt.int32),
                    0x7FFFFFFF, None, op.bitwise_and)
                nc.vector.tensor_scalar(
                    absR[:, 2 * NPIX:4 * NPIX].bitcast(dt.int32),
                    fieldB[:].bitcast(dt.int32),
                    0x7FFFFFFF, None, op.bitwise_and)
                # per-field sigma(kh - |R|): kh as per-partition ACT bias.
                # Field order A1 (pred w), A2 (tgt w), B1 (pred h), B2 (tgt h)
                sig = work.tile([128, 4 * NPIX], dt.bfloat16, tag="sig", bufs=3)
                for f, kh in ((0, pb("khw")[:, n:n + 1]), (1, tq(j, "khw")),
                              (2, pb("khh")[:, n:n + 1]), (3, tq(j, "khh"))):
                    nc.scalar.activation(
                        sig[:, f * NPIX:(f + 1) * NPIX],
                        absR[:, f * NPIX:(f + 1) * NPIX],
                        AF.Sigmoid, bias=kh, scale=-1.0)

                # F1 = sigA1*sigB1, F2 = sigA2*sigB2 on DVE; F1*F2 on Pool;
                # Isum reduced by ACT via a Copy activation's accum_out
                # F1 = sigA1*sigB1, F2 = sigA2*sigB2 AND their sum in one
                # native DVE scalar_tensor_tensor with accum_out (the ucode
                # tensor_tensor_reduce crashes on HW; this one passes).
                # F1*F2 on Pool, Isum via ACT Copy-accum.
                Fp = work.tile([128, 2 * NPIX], dt.bfloat16, tag="Fp", bufs=3)
                nc.vector.tensor_mul(Fp[:], sig[:, 0:2 * NPIX], sig[:, 2 * NPIX:4 * NPIX])
                Fpc = work.tile([128, 2 * NPIX], dt.bfloat16, tag="Fpc", bufs=3)
                nc.scalar.activation(Fpc[:], Fp[:], AF.Copy,
                                     accum_out=Ssum[:, n:n + 1])
                F12 = work.tile([128, NPIX], dt.bfloat16, tag="F12", bufs=3)
                nc.vector.scalar_tensor_tensor(
                    F12[:], Fp[:, 0:NPIX], 1.0, Fp[:, NPIX:2 * NPIX],
                    op.mult, op.mult, accum_out=Isum[:, n:n + 1])

            # ---- epilogue: piou = inter / (stot - inter + eps) ----
            union = scratch.tile([128, NLOC], dt.float32, tag="union")
            nc.vector.scalar_tensor_tensor(
                union[:], Isum[:], -1.0, Ssum[:], op.mult, op.add)
            nc.vector.tensor_scalar(union[:], union[:], float(EPS), None, op.add)
            rec = scratch.tile([128, NLOC], dt.float32, tag="rec")
            nc.vector.reciprocal(rec[:], union[:])
            piou = accp.tile([128, NLOC], dt.float16, tag="piou")
            nc.vector.tensor_tensor(piou[:], Isum[:], rec[:], op.mult)
            nc.sync.dma_start(out=OUT_d[j * 128:(j + 1) * 128, :], in_=piou[:])

    nc.finalize()
    return nc


def _get_runner():
    """Build (once) a reusable jitted PJRT dispatch for the Bass module.

    Re-jitting per call (what run_bass_kernel_spmd does under axon) costs
    ~400ms of retrace + XLA recompile every call; here the jitted callable,
    the device-resident BASIS constant, and the structural zero-output
    buffers all persist across calls.
    """
    if "runner" in _cache:
        return _cache["runner"]

    import jax
    from jax.sharding import Mesh, NamedSharding, PartitionSpec
    from jax.experimental.shard_map import shard_map  # check_rep kwarg
    from concourse import mybir
    from concourse.bass2jax import (
        _bass_exec_p,
        install_neuronx_cc_hook,
        partition_id_tensor,
    )

    nc = _build_nc()
    install_neuronx_cc_hook()
    partition_name = nc.partition_id_tensor.name if nc.partition_id_tensor else None

    in_names, out_names, out_avals = [], [], []
    for alloc in nc.m.functions[0].allocations:
        if not isinstance(alloc, mybir.MemoryLocationSet):
            continue
        name = alloc.memorylocations[0].name
        if alloc.kind == "ExternalInput":
            if name != partition_name:
                in_names.append(name)
        elif alloc.kind == "ExternalOutput":
            out_names.append(name)
            out_avals.append(
                jax.core.ShapedArray(tuple(alloc.tensor_shape),
                                     mybir.dt.np(alloc.dtype)))
    assert in_names == ["X", "BASIS"] and out_names == ["OUT"], (in_names, out_names)

    all_in_names = list(in_names) + list(out_names)
    if partition_name is not None:
        all_in_names.append(partition_name)

    def _body(*args):
        operands = list(args)
        if partition_name is not None:
            operands.append(partition_id_tensor())
        return tuple(_bass_exec_p.bind(
            *operands,
            out_avals=tuple(out_avals),
            in_names=tuple(all_in_names),
            out_names=tuple(out_names),
            lowering_input_output_aliases=(),
            sim_require_finite=True,
            sim_require_nnan=True,
            nc=nc,
        ))

    mesh = Mesh(np.asarray(jax.devices()[:NC]), ("core",))
    spec = PartitionSpec("core")
    n_ops = len(in_names) + len(out_names)
    fn = jax.jit(
        shard_map(_body, mesh=mesh, in_specs=(spec,) * n_ops,
                  out_specs=(spec,) * len(out_names), check_rep=False),
        keep_unused=True,
    )

    sh = NamedSharding(mesh, spec)
    # Device-resident across calls: the constant basis (replicated per core)
    # and the zero buffer backing the OUT operand slot.  The NEFF never
    # reads or writes this operand (the renamed NEFF binds OUT only as
    # output0, and the kernel writes every OUT element), so one buffer can
    # be reused every call without donation.
    bas_dev = jax.device_put(
        np.broadcast_to(_basis(), (NC, 12, 4 * NPIX)).reshape(NC * 12, 4 * NPIX), sh)
    zout_dev = jax.device_put(np.zeros((NC * M, NLOC), np.float16), sh)

    def run(x_packed):
        try:
            out, = fn(x_packed, bas_dev, zout_dev)
            return np.asarray(out)  # [NC*M, NLOC] fp16
        except Exception:
            # One retry for transient tunnel/runtime hiccups; a persistent
            # device wedge will re-raise.
            out, = fn(x_packed, bas_dev, zout_dev)
            return np.asarray(out)

    # Warm: pay NEFF compile + jit trace + device layout here, not in the
    # first timed call.
    run(np.zeros((NC * XROWS, 40), np.float32))

    _cache["runner"] = run
    return run


def kernel(loc_p, loc_t, grid):
    assert int(grid) == G
    loc_p = np.asarray(loc_p, dtype=np.float32)
    loc_t = np.asarray(loc_t, dtype=np.float32)

    run = _get_runner()
    res = run(_pack_inputs(loc_p, loc_t))  # [NC*M, NLOC] fp16
    # res[c*M + m, n] = piou(box c*NLOC+n, box m)  ->  out[n_global, m]
    out = res.reshape(NC, M, NLOC).transpose(0, 2, 1).reshape(N, M)
    return np.ascontiguousarray(out, dtype=np.float32)


# revision 47
# speedup vs baseline: 1.9173x; 1.9173x over previous
"""PIoU (pixel-wise IoU) pairwise matrix kernel for Trainium2, 8 NeuronCores.

Math: for each pair (predicted box n, target box m) the reference samples a
16x16 grid of the joint AABB and evaluates a soft membership
F = sigmoid(k(w/2-|A|)) * sigmoid(k(h/2-|B|)) per box, where (A, B) are the
pixel offsets rotated into the box frame.  Both A and B are *affine* in the
grid coordinates (ug, uh), so a K=12 fp32 matmul per (box, 128-target
chunk) against a constant (1, Ug, Uh) basis emits R = K*A for all 256
pixels x 4 fields; |R| is one DVE sign-bit clear, and ACT evaluates
sigma(kh - |R|) with the per-field kh as per-partition bias (scale=-1).
Products and reductions are spread over DVE (F1,F2 + Ssum), Pool (F1*F2),
and ACT (Isum via a Copy activation's accum_out) so the PE stays busy.

Sharding: N (predicted) axis split 8 ways; each core computes a [512m, 64n]
slab (output transposed on host).

Dispatch: the per-call cost is dominated by the axon tunnel round trip
(~80ms) plus payload bytes, so the runner (a) jits the PJRT shard_map
function ONCE and reuses it every call, (b) keeps the constant basis and
the structural zero-output buffers resident on device, and (c) ships only
one packed [144,40] f32 tensor per core per call (~184KB total) with the
per-box broadcast done on device via a K=1 matmul against an all-ones
basis row.  Output returns as fp16 (512KB) and is upcast on host.
"""

import numpy as np

N = 512
M = 512
G = 16
NPIX = G * G
K_SLOPE = np.float32(10.0)
EPS = np.float32(1e-6)
NC = 8
NLOC = N // NC  # 64 predicted boxes per core
NCHUNK = 4  # m-chunks of 128
XROWS = 144  # packed input: rows 0..127 = TQ [128,40], rows 128..143 = PB [16,40]

_cache = {}

_Q_ORDER = ("x0", "x1", "y0", "y1", "cx", "cy", "ct", "st", "khw", "khh")


def _derived(b):
    # b: [K,5] float32 -> per-box derived quantities (all float32)
    cx, cy, w, h, t = (b[:, i].astype(np.float32) for i in range(5))
    c, s = np.cos(t).astype(np.float32), np.sin(t).astype(np.float32)
    hw = np.float32(0.5) * (w * np.abs(c) + h * np.abs(s))
    hh = np.float32(0.5) * (w * np.abs(s) + h * np.abs(c))
    return dict(
        cx=cx, cy=cy, ct=c, st=s,
        khw=(K_SLOPE * np.float32(0.5)) * w, khh=(K_SLOPE * np.float32(0.5)) * h,
        x0=cx - hw, x1=cx + hw, y0=cy - hh, y1=cy + hh,
    )


def _basis():
    """Constant [12, NPIX*4] sampling basis (shipped to device once).

    Field f (A1, A2, B1, B2) occupies cols f*NPIX..(f+1)*NPIX and uses rows
    3f..3f+2 as (1, Ug, Uh).  The matmul emits R = K*(a0 + a1*ug + a2*uh)
    per field; |R| (sign-bit clear on DVE) then feeds sigma(kh - |R|) on
    ACT via per-field per-partition biases.  Row 0 is all-ones over cols
    0..255; its first 128 cols double as the ones-lhsT for the on-device
    partition broadcast of the per-box rows.
    """
    if "basis" in _cache:
        return _cache["basis"]
    u = ((np.arange(G, dtype=np.float32) + np.float32(0.5)) / np.float32(G))
    Ug = np.tile(u, G)      # pixel p = h*G+g -> u[g]
    Uh = np.repeat(u, G)    # -> u[h]

    basis = np.zeros((12, 4 * NPIX), dtype=np.float32)
    for f in range(4):
        c0 = f * NPIX
        basis[3 * f + 0, c0:c0 + NPIX] = 1.0
        basis[3 * f + 1, c0:c0 + NPIX] = Ug
        basis[3 * f + 2, c0:c0 + NPIX] = Uh
    _cache["basis"] = basis
    return basis


def _pack_inputs(loc_p, loc_t):
    """Per-call packed input [NC*XROWS, 40]: TQ (replicated) + per-core PB."""
    T = _derived(loc_t)
    # TQ [128, 4 chunks, 10]: per-target quantities, m = j*128 + partition
    TQ = np.empty((128, NCHUNK, len(_Q_ORDER)), dtype=np.float32)
    for qi, q in enumerate(_Q_ORDER):
        TQ[:, :, qi] = T[q].reshape(NCHUNK, 128).T
    TQflat = TQ.reshape(128, NCHUNK * len(_Q_ORDER))

    P = _derived(loc_p)
    X = np.empty((NC, XROWS, 40), dtype=np.float32)
    for c in range(NC):
        sl = slice(c * NLOC, (c + 1) * NLOC)
        pb = np.stack([P[q][sl] for q in _Q_ORDER], axis=0)  # [10, 64]
        X[c, 0:128, :] = TQflat
        X[c, 128:XROWS, :] = pb.reshape(16, 40)
    return X.reshape(NC * XROWS, 40)


def _build_nc():
    from contextlib import ExitStack

    import concourse.bacc as bacc
    import concourse.tile as tile
    from concourse import mybir
    from concourse.masks import make_identity

    dt = mybir.dt
    op = mybir.AluOpType
    AF = mybir.ActivationFunctionType
    K = float(K_SLOPE)

    # Bacc (not raw Bass): its finalize() runs generate_event_semaphores,
    # which legalizes Tile's multi-wait sync_info down to <=1 wait per
    # hardware instruction.
    nc = bacc.Bacc(None, target_bir_lowering=False)
    X_d = nc.declare_dram_parameter("X", [XROWS, 40], dt.float32, isOutput=False)
    BAS_d = nc.declare_dram_parameter("BASIS", [12, 4 * NPIX], dt.float32, isOutput=False)
    OUT_d = nc.declare_dram_parameter("OUT", [M, NLOC], dt.float16, isOutput=True)

    with tile.TileContext(nc) as tc, ExitStack() as ctx:
        consts = ctx.enter_context(tc.tile_pool(name="consts", bufs=1))
        coeffp = ctx.enter_context(tc.tile_pool(name="coeffp", bufs=2))
        scratch = ctx.enter_context(tc.tile_pool(name="scratch", bufs=2))
        work = ctx.enter_context(tc.tile_pool(name="work", bufs=2))
        accp = ctx.enter_context(tc.tile_pool(name="accp", bufs=2))
        psum = ctx.enter_context(tc.tile_pool(name="psum", bufs=2, space="PSUM"))

        ident = consts.tile([128, 128], dt.float32)
        make_identity(nc, ident[:])
        TQ = consts.tile([128, NCHUNK, 10], dt.float32)
        nc.sync.dma_start(out=TQ[:].rearrange("p a b -> p (a b)"), in_=X_d[0:128, :])
        pbrow = consts.tile([1, 10 * NLOC], dt.float32)
        nc.sync.dma_start(
            out=pbrow[:],
            in_=X_d[128:XROWS, :].rearrange("a b -> (a b)").unsqueeze(0))
        BAS = consts.tile([12, 4 * NPIX], dt.float32)
        nc.sync.dma_start(out=BAS[:], in_=BAS_d[:])

        # Broadcast the 640 per-box values to all 128 partitions with a K=1
        # matmul: BAS row 0 is all-ones on cols 0..255, so BAS[0:1, 0:128]
        # is a ready-made ones lhsT.  PSUM reuses one "fields" ring slot.
        PB = consts.tile([128, 10, NLOC], dt.float32)
        PBflat = PB[:].rearrange("p a b -> p (a b)")
        bc = psum.tile([128, 2 * NPIX], dt.float32, tag="fieldA", bufs=3)
        nc.tensor.matmul(bc[:], BAS[0:1, 0:128],
                         pbrow[:, 0:512].broadcast_to([1, 512]),
                         start=True, stop=True)
        bc2 = psum.tile([128, 2 * NPIX], dt.float32, tag="fieldB", bufs=3)
        nc.tensor.matmul(bc2[:, 0:128], BAS[0:1, 0:128], pbrow[:, 512:640],
                         start=True, stop=True)
        nc.vector.tensor_copy(PBflat[:, 0:512], bc[:])
        nc.vector.tensor_copy(PBflat[:, 512:640], bc2[:, 0:128])

        def pb(q):
            return PB[:, _Q_ORDER.index(q), :]

        def tq(j, q):
            i = _Q_ORDER.index(q)
            return TQ[:, j, i:i + 1]

        for j in range(NCHUNK):
            # ---- coefficient slab C [128 m, 64 n, 12 rows] on DVE ----
            C = coeffp.tile([128, NLOC, 12], dt.float32, tag="C")
            S = scratch.tile([128, 16, NLOC], dt.float32, tag="S")
            g = nc.vector

            def s(i):
                return S[:, i, :]

            g.tensor_scalar(s(0), pb("x0"), tq(j, "x0"), None, op.min)   # xmin
            g.tensor_scalar(s(1), pb("x1"), tq(j, "x1"), None, op.max)   # xmax
            g.tensor_scalar(s(2), pb("y0"), tq(j, "y0"), None, op.min)   # ymin
            g.tensor_scalar(s(3), pb("y1"), tq(j, "y1"), None, op.max)   # ymax
            g.tensor_tensor(s(4), s(1), s(0), op.subtract)               # sx
            g.tensor_tensor(s(5), s(3), s(2), op.subtract)               # sy
            g.tensor_tensor(s(6), s(0), pb("cx"), op.subtract)           # dxp
            g.tensor_tensor(s(7), s(2), pb("cy"), op.subtract)           # dyp
            # a0p = dxp*ctp + dyp*stp ; b0p = dyp*ctp - dxp*stp
            g.tensor_tensor(s(8), s(6), pb("ct"), op.mult)
            g.tensor_tensor(s(9), s(7), pb("st"), op.mult)
            g.tensor_tensor(s(9), s(8), s(9), op.add)                    # a0p
            g.tensor_tensor(s(8), s(7), pb("ct"), op.mult)
            g.tensor_tensor(s(10), s(6), pb("st"), op.mult)
            g.tensor_tensor(s(10), s(8), s(10), op.subtract)             # b0p

            def c(r):
                return C[:, :, r]

            # R-rows: field f cols use basis rows 3f..3f+2 as (1, Ug, Uh);
            # coefficient rows are K*(a0, a1, a2) per field.
            # field A1 (pred w-axis): K*a0p, K*a1p (=K*sx*ctp), K*a2p (=K*sy*stp)
            g.tensor_scalar(c(0), s(9), K, None, op.mult)
            g.tensor_tensor(s(8), s(4), pb("ct"), op.mult)
            g.tensor_scalar(c(1), s(8), K, None, op.mult)
            g.tensor_tensor(s(8), s(5), pb("st"), op.mult)
            g.tensor_scalar(c(2), s(8), K, None, op.mult)
            # field B1 (pred h-axis, rows 6-8): K*b0p, -K*sx*stp, K*sy*ctp
            g.tensor_scalar(c(6), s(10), K, None, op.mult)
            g.tensor_tensor(s(8), s(4), pb("st"), op.mult)
            g.tensor_scalar(c(7), s(8), -K, None, op.mult)
            g.tensor_tensor(s(8), s(5), pb("ct"), op.mult)
            g.tensor_scalar(c(8), s(8), K, None, op.mult)
            # target box: dxt/dyt
            g.tensor_scalar(s(12), s(0), tq(j, "cx"), None, op.subtract)
            g.tensor_scalar(s(13), s(2), tq(j, "cy"), None, op.subtract)
            # a0t = dxt*ctt + dyt*stt
            g.tensor_scalar(s(8), s(12), tq(j, "ct"), None, op.mult)
            g.tensor_scalar(s(14), s(13), tq(j, "st"), None, op.mult)
            g.tensor_tensor(s(14), s(8), s(14), op.add)
            # b0t = dyt*ctt - dxt*stt
            g.tensor_scalar(s(8), s(13), tq(j, "ct"), None, op.mult)
            g.tensor_scalar(s(15), s(12), tq(j, "st"), None, op.mult)
            g.tensor_tensor(s(15), s(8), s(15), op.subtract)
            # field A2 (tgt w-axis, rows 3-5): K*a0t, K*sx*ctt, K*sy*stt
            g.tensor_scalar(c(3), s(14), K, None, op.mult)
            g.tensor_scalar(s(8), s(4), tq(j, "ct"), None, op.mult)
            g.tensor_scalar(c(4), s(8), K, None, op.mult)
            g.tensor_scalar(s(8), s(5), tq(j, "st"), None, op.mult)
            g.tensor_scalar(c(5), s(8), K, None, op.mult)
            # field B2 (tgt h-axis, rows 9-11): K*b0t, -K*sx*stt, K*sy*ctt
            g.tensor_scalar(c(9), s(15), K, None, op.mult)
            g.tensor_scalar(s(8), s(4), tq(j, "st"), None, op.mult)
            g.tensor_scalar(c(10), s(8), -K, None, op.mult)
            g.tensor_scalar(s(8), s(5), tq(j, "ct"), None, op.mult)
            g.tensor_scalar(c(11), s(8), K, None, op.mult)

            Ssum = accp.tile([128, NLOC], dt.float32, tag="Ssum")
            Isum = accp.tile([128, NLOC], dt.float32, tag="Isum")

            # ---- main loop over the 64 predicted boxes of this core ----
            for n in range(NLOC):
                coeffT = psum.tile([12, 128], dt.float32, tag="coeffT")
                nc.tensor.transpose(coeffT[:], C[:, n, :], ident[:])
                lhsT = work.tile([12, 128], dt.float32, tag="lhsT", bufs=3)
                nc.vector.tensor_copy(lhsT[:], coeffT[:])

                fieldA = psum.tile([128, 2 * NPIX], dt.float32, tag="fieldA", bufs=3)
                fieldB = psum.tile([128, 2 * NPIX], dt.float32, tag="fieldB", bufs=3)
                nc.tensor.matmul(fieldA[:], lhsT[:], BAS[:, 0:512],
                                 start=True, stop=True)
                nc.tensor.matmul(fieldB[:], lhsT[:], BAS[:, 512:1024],
                                 start=True, stop=True)
                # |R| by clearing the sign bit (single PSUM read; GPSIMD
                # cannot access PSUM and DVE allows only one PSUM input).
                # Two halves so the A-half abs overlaps the B-half matmul.
                absR = work.tile([128, 4 * NPIX], dt.float32, tag="absR", bufs=3)
                nc.vector.tensor_scalar(
                    absR[:, 0:2 * NPIX].bitcast(dt.int32),
                    fieldA[:].bitcast(dt.int32),
                    0x7FFFFFFF, None, op.bitwise_and)
                nc.vector.tensor_scalar(
                    absR[:, 2 * NPIX:4 * NPIX].bitcast(dt.int32),
                    fieldB[:].bitcast(dt.int32),
                    0x7FFFFFFF, None, op.bitwise_and)
                # per-field sigma(kh - |R|): kh as per-partition ACT bias.
                # Field order A1 (pred w), A2 (tgt w), B1 (pred h), B2 (tgt h)
                sig = work.tile([128, 4 * NPIX], dt.bfloat16, tag="sig", bufs=3)
                for f, kh in ((0, pb("khw")[:, n:n + 1]), (1, tq(j, "khw")),
                              (2, pb("khh")[:, n:n + 1]), (3, tq(j, "khh"))):
                    nc.scalar.activation(
                        sig[:, f * NPIX:(f + 1) * NPIX],
                        absR[:, f * NPIX:(f + 1) * NPIX],
                        AF.Sigmoid, bias=kh, scale=-1.0)

                # F1 = sigA1*sigB1, F2 = sigA2*sigB2 on DVE; F1*F2 on Pool;
                # Isum reduced by ACT via a Copy activation's accum_out
                # F1 = sigA1*sigB1, F2 = sigA2*sigB2 AND their sum in one
                # native DVE scalar_tensor_tensor with accum_out (the ucode
                # tensor_tensor_reduce crashes on HW; this one passes).
                # F1*F2 on Pool, Isum via ACT Copy-accum.
                Fp = work.tile([128, 2 * NPIX], dt.bfloat16, tag="Fp", bufs=3)
                nc.vector.tensor_mul(Fp[:], sig[:, 0:2 * NPIX], sig[:, 2 * NPIX:4 * NPIX])
                Fpc = work.tile([128, 2 * NPIX], dt.bfloat16, tag="Fpc", bufs=3)
                nc.scalar.activation(Fpc[:], Fp[:], AF.Copy,
                                     accum_out=Ssum[:, n:n + 1])
                F12 = work.tile([128, NPIX], dt.bfloat16, tag="F12", bufs=3)
                nc.vector.scalar_tensor_tensor(
                    F12[:], Fp[:, 0:NPIX], 1.0, Fp[:, NPIX:2 * NPIX],
                    op.mult, op.mult, accum_out=Isum[:, n:n + 1])

            # ---- epilogue: piou = inter / (stot - inter + eps) ----
            union = scratch.tile([128, NLOC], dt.float32, tag="union")
            nc.vector.scalar_tensor_tensor(
                union[:], Isum[:], -1.0, Ssum[:], op.mult, op.add)
            nc.vector.tensor_scalar(union[:], union[:], float(EPS), None, op.add)
            rec = scratch.tile([128, NLOC], dt.float32, tag="rec")
            nc.vector.reciprocal(rec[:], union[:])
            piou = accp.tile([128, NLOC], dt.float16, tag="piou")
            nc.vector.tensor_tensor(piou[:], Isum[:], rec[:], op.mult)
            nc.sync.dma_start(out=OUT_d[j * 128:(j + 1) * 128, :], in_=piou[:])

    nc.finalize()
    return nc


def _get_runner():
    """Build (once) a reusable jitted PJRT dispatch for the Bass module.

    Re-jitting per call (what run_bass_kernel_spmd does under axon) costs
    ~400ms of retrace + XLA recompile every call; here the jitted callable,
    the device-resident BASIS constant, and the structural zero-output
    buffers all persist across calls.
    """
    if "runner" in _cache:
        return _cache["runner"]

    import jax
    from jax.sharding import Mesh, NamedSharding, PartitionSpec
    from jax.experimental.shard_map import shard_map  # check_rep kwarg
    from concourse import mybir
    from concourse.bass2jax import (
        _bass_exec_p,
        install_neuronx_cc_hook,
        partition_id_tensor,
    )

    nc = _build_nc()
    install_neuronx_cc_hook()
    partition_name = nc.partition_id_tensor.name if nc.partition_id_tensor else None

    in_names, out_names, out_avals = [], [], []
    for alloc in nc.m.functions[0].allocations:
        if not isinstance(alloc, mybir.MemoryLocationSet):
            continue
        name = alloc.memorylocations[0].name
        if alloc.kind == "ExternalInput":
            if name != partition_name:
                in_names.append(name)
        elif alloc.kind == "ExternalOutput":
            out_names.append(name)
            out_avals.append(
                jax.core.ShapedArray(tuple(alloc.tensor_shape),
                                     mybir.dt.np(alloc.dtype)))
    assert in_names == ["X", "BASIS"] and out_names == ["OUT"], (in_names, out_names)

    all_in_names = list(in_names) + list(out_names)
    if partition_name is not None:
        all_in_names.append(partition_name)

    def _body(*args):
        operands = list(args)
        if partition_name is not None:
            operands.append(partition_id_tensor())
        return tuple(_bass_exec_p.bind(
            *operands,
            out_avals=tuple(out_avals),
            in_names=tuple(all_in_names),
            out_names=tuple(out_names),
            lowering_input_output_aliases=(),
            sim_require_finite=True,
            sim_require_nnan=True,
            nc=nc,
        ))

    mesh = Mesh(np.asarray(jax.devices()[:NC]), ("core",))
    spec = PartitionSpec("core")
    n_ops = len(in_names) + len(out_names)
    fn = jax.jit(
        shard_map(_body, mesh=mesh, in_specs=(spec,) * n_ops,
                  out_specs=(spec,) * len(out_names), check_rep=False),
        keep_unused=True,
    )

    sh = NamedSharding(mesh, spec)
    # Device-resident across calls: the constant basis (replicated per core)
    # and the zero buffer backing the OUT operand slot.  The NEFF never
    # reads or writes this operand (the renamed NEFF binds OUT only as
    # output0, and the kernel writes every OUT element), so one buffer can
    # be reused every call without donation.
    bas_dev = jax.device_put(
        np.broadcast_to(_basis(), (NC, 12, 4 * NPIX)).reshape(NC * 12, 4 * NPIX), sh)
    zout_dev = jax.device_put(np.zeros((NC * M, NLOC), np.float16), sh)

    def run(x_packed):
        try:
            out, = fn(x_packed, bas_dev, zout_dev)
            return np.asarray(out)  # [NC*M, NLOC] fp16
        except Exception:
            # One retry for transient tunnel/runtime hiccups; a persistent
            # device wedge will re-raise.
            out, = fn(x_packed, bas_dev, zout_dev)
            return np.asarray(out)

    # Warm: pay NEFF compile + jit trace + device layout here, not in the
    # first timed call.
    run(np.zeros((NC * XROWS, 40), np.float32))

    _cache["runner"] = run
    return run


def kernel(loc_p, loc_t, grid):
    assert int(grid) == G
    loc_p = np.asarray(loc_p, dtype=np.float32)
    loc_t = np.asarray(loc_t, dtype=np.float32)

    run = _get_runner()
    res = run(_pack_inputs(loc_p, loc_t))  # [NC*M, NLOC] fp16
    # res[c*M + m, n] = piou(box c*NLOC+n, box m)  ->  out[n_global, m]
    out = res.reshape(NC, M, NLOC).transpose(0, 2, 1).reshape(N, M)
    return np.ascontiguousarray(out, dtype=np.float32)
